# revision 47
# baseline (speedup 1.0000x reference)
"""Trainium2 Bass kernel for blocksparse (sink+local) Llama attention.

Sharding: tensor-parallel by head across 8 NeuronCores. Core c computes
q-heads [4c, 4c+4) and kv-head c (the matching GQA group):
  - q/k/v projections column-parallel (per-core weight slices)
  - RoPE + blocksparse streaming attention fully head-local
  - o_proj row-parallel: each core emits a partial [S, HID] product
The row-parallel all-reduce is done at unshard time on the host (an 8-way
fp32 sum), which is far cheaper than an on-device collective here.

v2: single merged software pipeline over 512-column chunks. Iteration c
projects chunk c (two-pass sweep over 32 contraction tiles, PSUM banks
shared through a 5-slot ring) while the attention group for chunk c-1's
four q-blocks runs behind it; o_proj is software-pipelined TWO q-blocks
back as PE filler during softmax. Attention fuses the 4 GQA heads into
512-wide matmuls (S^T = K_j^T [q_h0|q_h1|q_h2|q_h3]); the causal mask
on the diagonal block is a 0/1 multiply on the exp output in SBUF (no
DVE-on-PSUM round trip). Softmax column sums: the granted blocks' P^T
tiles are folded by an fp32 add chain on the Vector engine (one bf16
rounding at the end), then a single all-ones stationary matmul per
q-block reduces across partitions (and broadcasts the sums for free);
each q-block's sum/reciprocal/normalize is deferred one q-block so the
fold never stalls the PE. o_proj PSUM->SBUF copies run on the Vector
engine, keeping the Scalar engine exclusively on exp (avoids
activation-table switches); q/attnT tiles are chunk-rotated q-block-major so
every matmul operand is a contiguous 2D access pattern.

Everything on device runs in bf16 with fp32 PSUM accumulation.
"""

import sys

sys.path.insert(0, "/opt/trn_rl_repo")

import math
from contextlib import ExitStack

import ml_dtypes
import numpy as np

import concourse.bass as bass
import concourse.tile as tile
from concourse import bacc, mybir
from concourse.masks import make_identity, make_upper_triangular

BF16 = mybir.dt.bfloat16
F32 = mybir.dt.float32
NPBF = ml_dtypes.bfloat16

N_CORES = 8
S = 4096
HID = 4096
NH, NKV, D = 32, 8, 128
QH = NH // N_CORES          # 4 q heads per core
BLK = 128
NB = S // BLK               # 32 blocks
LOCAL_NB = 8
SCHUNK = 512                # s-columns processed per pipeline iteration
NSC = S // SCHUNK           # 8
QBC = SCHUNK // BLK         # 4 q-blocks per chunk
HT = HID // 128             # 32 contraction tiles
GRP = 8                     # contraction tiles per grouped hs DMA
THETA = 10000.0
EXPF = mybir.ActivationFunctionType.Exp

import os

SUMS_MODE = os.environ.get("K_SUMS_MODE", "dve")   # pe | dve | gpsimd
OB_ENGINE = os.environ.get("K_OB_ENGINE", "dve")   # dve | act


def _rope_into(nc, pool, dst, ps, cos_c, sin_c):
    """dst(bf16) = ps * cos_c + swap_halves(ps) * sin_c  (sin_c sign-baked).

    ps is a [128, SCHUNK] fp32 PSUM tile holding a projection output d-block;
    partition p is feature dim d. swap_halves pairs d <-> d+64.
    dst may be a 3D AP (q-block-major scatter); its free size must be SCHUNK.
    """
    t0 = pool.tile([128, SCHUNK], F32, tag="rope_t0", name="rope_t0")
    t1 = pool.tile([128, SCHUNK], F32, tag="rope_t1", name="rope_t1")
    nc.vector.tensor_mul(t0, ps, cos_c)
    nc.vector.tensor_mul(t1[0:64, :], ps[64:128, :], sin_c[0:64, :])
    nc.vector.tensor_mul(t1[64:128, :], ps[0:64, :], sin_c[64:128, :])
    if len(dst.shape) == 3:
        nqb = dst.shape[1]
        nc.vector.tensor_add(
            dst,
            t0.rearrange("p (a b) -> p a b", a=nqb),
            t1.rearrange("p (a b) -> p a b", a=nqb),
        )
    else:
        nc.vector.tensor_add(dst, t0, t1)


def _emit_body(nc, tc, persist, aps):
    hsT, wq, wk, wv, wo, cos2d, sin2d, out_p = aps

    kT = persist.tile([128, S], BF16, name="kT")           # [d | s]
    vN = persist.tile([128, S], BF16, name="vN")           # [s_in_blk | (blk, d)]
    wq_sb = persist.tile([128, HT * QH * 128], BF16, name="wq_sb")
    wk_sb = persist.tile([128, HT * 128], BF16, name="wk_sb")
    wv_sb = persist.tile([128, HT * 128], BF16, name="wv_sb")
    wo_sb = persist.tile([128, QH * HID], BF16, name="wo_sb")
    cos_sb = persist.tile([128, S], BF16, name="cos_sb")
    sin_sb = persist.tile([128, S], BF16, name="sin_sb")
    ones = persist.tile([128, 128], BF16, name="ones")
    tri01 = persist.tile([128, 128], BF16, name="tri01")   # 1 where k <= q
    ident = persist.tile([128, 128], BF16, name="ident")

    nc.vector.memset(ones, 1.0)
    make_upper_triangular(nc, tri01, val=1.0, diag=True)
    make_identity(nc, ident)

    st = ExitStack()
    hs_pool = st.enter_context(tc.tile_pool(name="hs_pool", bufs=5))
    qT_pool = st.enter_context(tc.tile_pool(name="qT_pool", bufs=2))
    aT_pool = st.enter_context(tc.tile_pool(name="aT_pool", bufs=2))
    pt_pool = st.enter_context(tc.tile_pool(name="pt_pool", bufs=2))
    rope_pool = st.enter_context(tc.tile_pool(name="rope_pool", bufs=1))
    vt_pool = st.enter_context(tc.tile_pool(name="vt_pool", bufs=2))
    ob_pool = st.enter_context(tc.tile_pool(name="ob_pool", bufs=5))
    rb_pool = st.enter_context(tc.tile_pool(name="rb_pool", bufs=2))
    fold_pool = st.enter_context(tc.tile_pool(name="fold_pool", bufs=2))
    ps_pool = st.enter_context(tc.tile_pool(name="ps_pool", bufs=5, space="PSUM"))
    ps_acc = st.enter_context(tc.tile_pool(name="ps_acc", bufs=1, space="PSUM"))

    # ---- one-time loads, split + ordered so the first matmuls start early:
    # wq arrives per contraction-tile group (the sweep consumes it in order),
    # wk/wv/tables before pass B / RoPE, and the big wo load last (first
    # o_proj is ~2 chunks in).
    def _load_wq_group(g):
        nc.sync.dma_start(
            out=wq_sb[:, g * GRP * 512 : (g + 1) * GRP * 512].rearrange(
                "p (t c) -> p t c", c=512
            ),
            in_=wq[g * GRP * 128 : (g + 1) * GRP * 128, :].rearrange(
                "(t p) c -> p t c", p=128
            ),
        )

    def _load_rest():
        nc.sync.dma_start(
            out=wk_sb.rearrange("p (t c) -> p t c", c=128),
            in_=wk.rearrange("(t p) c -> p t c", p=128),
        )
        nc.sync.dma_start(
            out=wv_sb.rearrange("p (t c) -> p t c", c=128),
            in_=wv.rearrange("(t p) c -> p t c", p=128),
        )
        nc.sync.dma_start(out=cos_sb, in_=cos2d)
        nc.sync.dma_start(out=sin_sb, in_=sin2d)
        nc.sync.dma_start(
            out=wo_sb.rearrange("p (t c) -> p t c", c=HID),
            in_=wo.rearrange("(t p) c -> p t c", p=128),
        )

    qTs = [None, None]    # chunk-rotated q tiles [128, QH*SCHUNK], (h, s)
    aTs = [None, None]    # chunk-rotated attention-out tiles, (h, s)

    def _load_chunk(sc, first=False):
        scol = slice(sc * SCHUNK, (sc + 1) * SCHUNK)
        grps = []
        for g in range(HT // GRP):
            if first:
                _load_wq_group(g)
            hg = hs_pool.tile([128, GRP * SCHUNK], BF16, tag="hs",
                              name=f"hs_{sc}_{g}")
            nc.sync.dma_start(
                out=hg.rearrange("p (t c) -> p t c", c=SCHUNK),
                in_=hsT[g * GRP * 128 : (g + 1) * GRP * 128, scol].rearrange(
                    "(t p) c -> p t c", p=128
                ),
            )
            grps.append(hg)
        if first:
            _load_rest()
        return grps

    def _hs_of(grps, ht):
        return grps[ht // GRP][:, (ht % GRP) * SCHUNK : (ht % GRP + 1) * SCHUNK]

    def _proj_chunk(sc, grps):
        """Project chunk sc. Two-pass sweep (q heads, then k+v) so the q PSUM
        banks are roped and freed while pass B runs, keeping the shared ring
        drained before the attention group needs it."""
        scol = slice(sc * SCHUNK, (sc + 1) * SCHUNK)
        cos_c = cos_sb[:, scol]
        sin_c = sin_sb[:, scol]
        qTc = qT_pool.tile([128, QH * SCHUNK], BF16, tag="qT", name=f"qT_{sc}")
        qTs[sc % 2] = qTc
        # pass A: q heads 0-2
        psqs = [ps_pool.tile([128, SCHUNK], F32, tag="ps", name=f"ps_q{db}")
                for db in range(3)]
        for ht in range(HT):
            hsv = _hs_of(grps, ht)
            for db in range(3):
                nc.tensor.matmul(
                    psqs[db],
                    lhsT=wq_sb[:, ht * 512 + db * 128 : ht * 512 + (db + 1) * 128],
                    rhs=hsv,
                    start=(ht == 0),
                    stop=(ht == HT - 1),
                )
        qT3 = qTc.rearrange("p (qb hq) -> p qb hq", qb=QBC)
        for db in range(3):
            _rope_into(nc, rope_pool, qT3[:, :, db * 128 : (db + 1) * 128],
                       psqs[db], cos_c, sin_c)
        # pass B: q head 3, k and v (pass A banks free via RoPE meanwhile)
        psq3 = ps_pool.tile([128, SCHUNK], F32, tag="ps", name="ps_q3")
        psk = ps_pool.tile([128, SCHUNK], F32, tag="ps", name="ps_k")
        psv = ps_pool.tile([128, SCHUNK], F32, tag="ps", name="ps_v")
        for ht in range(HT):
            hsv = _hs_of(grps, ht)
            nc.tensor.matmul(
                psq3,
                lhsT=wq_sb[:, ht * 512 + 3 * 128 : ht * 512 + 4 * 128],
                rhs=hsv,
                start=(ht == 0), stop=(ht == HT - 1),
            )
            nc.tensor.matmul(
                psk, lhsT=wk_sb[:, ht * 128 : (ht + 1) * 128], rhs=hsv,
                start=(ht == 0), stop=(ht == HT - 1),
            )
            nc.tensor.matmul(
                psv, lhsT=wv_sb[:, ht * 128 : (ht + 1) * 128], rhs=hsv,
                start=(ht == 0), stop=(ht == HT - 1),
            )
        _rope_into(nc, rope_pool, qT3[:, :, 3 * 128 : 4 * 128],
                   psq3, cos_c, sin_c)
        _rope_into(nc, rope_pool, kT[:, scol], psk, cos_c, sin_c)
        # v: PSUM -> SBUF copy (ACT); block transposes are deferred until
        # after the attention group so they don't contend for PSUM slots
        vt_sb = vt_pool.tile([128, SCHUNK], BF16, tag="vt_sb", name="vt_sb")
        nc.scalar.copy(out=vt_sb, in_=psv)
        return vt_sb

    def _v_transpose(sc, vt_sb):
        for b in range(QBC):
            pvt = ps_pool.tile([128, 128], BF16, tag="ps", name="pvt")
            nc.tensor.transpose(pvt, vt_sb[:, b * 128 : (b + 1) * 128], ident)
            nc.vector.tensor_copy(
                vN[:, (sc * QBC + b) * 128 : (sc * QBC + b + 1) * 128], pvt)

    def _oproj(i):
        """o_proj for q-block i (reads chunk-rotated attnT tile)."""
        aTc = aTs[(i // QBC) % 2]
        qb = i % QBC
        for cg in range(HID // 512):
            ps = ps_pool.tile([128, 512], F32, tag="ps", name="ps_o")
            for h in range(QH):
                nc.tensor.matmul(
                    ps,
                    lhsT=aTc[:, qb * 512 + h * 128 : qb * 512 + (h + 1) * 128],
                    rhs=wo_sb[:, h * HID + cg * 512 : h * HID + (cg + 1) * 512],
                    start=(h == 0),
                    stop=(h == QH - 1),
                )
            ob = ob_pool.tile([128, 512], BF16, tag="ob", name="ob")
            if OB_ENGINE == "act":
                nc.scalar.copy(out=ob, in_=ps)
            else:
                nc.vector.tensor_copy(ob, ps)
            nc.sync.dma_start(
                out=out_p[i * 128 : (i + 1) * 128, cg * 512 : (cg + 1) * 512],
                in_=ob,
            )

    pending = [None]   # (i, folded, O_ps) awaiting sum/recip/normalize
    sum_tiles = [None]

    def _finish_pending():
        if pending[0] is None:
            return
        i, folded, O_ps = pending[0]
        pending[0] = None
        qb = i % QBC
        sum_ps = ps_acc.tile([128, 512], F32, tag="sum", name="sum_ps")
        nc.tensor.matmul(sum_ps, lhsT=ones, rhs=folded, start=True, stop=True)
        rb = rb_pool.tile([128, 512], F32, tag="rb", name="rb")
        nc.vector.reciprocal(rb, sum_ps)
        aTc = aTs[(i // QBC) % 2]
        nc.vector.tensor_mul(
            aTc[:, qb * 512 : (qb + 1) * 512], O_ps, rb)

    def _attn_qblock(i):
        """Attention for q-block i, 4 heads fused (512-wide matmuls)."""
        qTc = qTs[(i // QBC) % 2]
        qb = i % QBC
        q4 = qTc[:, qb * 512 : (qb + 1) * 512]   # [128, 512], (h, q) cols
        # block order: diagonal first (so its exp+mask clears early),
        # then sink, then the remaining local blocks
        if i == 0:
            blocks = [0]
        else:
            L = min(i, LOCAL_NB)
            js = i - L + 1
            blocks = [i, 0] + list(range(js, i))
        nblk = len(blocks)

        PT = pt_pool.tile([128, 9 * 512], BF16, tag="PT", name="PT")
        s_tiles = []
        for bi, j in enumerate(blocks):
            s_ps = ps_pool.tile([128, 512], F32, tag="ps", name="s_ps")
            nc.tensor.matmul(
                s_ps, lhsT=kT[:, j * 128 : (j + 1) * 128], rhs=q4,
                start=True, stop=True,
            )
            s_tiles.append(s_ps)
            nc.scalar.activation(
                out=PT[:, bi * 512 : (bi + 1) * 512], in_=s_ps, func=EXPF)
        # token-causal 0/1 mask on the diagonal block (per head)
        for h in range(QH):
            nc.vector.tensor_mul(
                PT[:, h * 128 : (h + 1) * 128],
                PT[:, h * 128 : (h + 1) * 128],
                tri01,
            )
        # Fold the granted blocks' P^T tiles on the (otherwise idle) GPSIMD
        # engine: fp32 accumulator chain, final add (with the masked
        # diagonal) rounds once to bf16. The finish step for this q-block
        # (one column-sum matmul + recip + normalize) is deferred to the
        # NEXT q-block's slot, so the serial GPSIMD chain has a full
        # iteration of slack and the PE never waits on it.
        def _pt(bi):
            return PT[:, bi * 512 : (bi + 1) * 512]

        folded = None
        if SUMS_MODE != "pe":
            eng = nc.gpsimd if SUMS_MODE == "gpsimd" else nc.vector
            if nblk == 1:
                folded = _pt(0)
            else:
                folded = fold_pool.tile([128, 512], BF16, tag="foldb",
                                        name="foldb")
                if nblk == 2:
                    eng.tensor_add(folded, _pt(1), _pt(0))
                else:
                    acc = fold_pool.tile([128, 512], F32, tag="foldf",
                                         name="foldf")
                    eng.tensor_add(acc, _pt(1), _pt(2))
                    for bi in range(3, nblk):
                        eng.tensor_add(acc, acc, _pt(bi))
                    eng.tensor_add(folded, acc, _pt(0))
        # finish the PREVIOUS q-block now that its fold had a full
        # iteration to complete
        _finish_pending()
        # PE filler while exp/mask/fold run: o_proj lagged TWO q-blocks so
        # its attnT input always has a full iteration of slack
        if i >= 2:
            _oproj(i - 2)
        # P^T @ V; diagonal block last (waits on the mask multiply)
        O_ps = ps_acc.tile([128, 512], F32, tag="O", name="O_ps", bufs=2)
        order = list(range(1, nblk)) + [0]
        for oi, bi in enumerate(order):
            j = blocks[bi]
            pts = PT[:, bi * 512 : (bi + 1) * 512]
            nc.tensor.matmul(
                O_ps, lhsT=vN[:, j * 128 : (j + 1) * 128], rhs=pts,
                start=(oi == 0), stop=(oi == nblk - 1),
            )
            if SUMS_MODE == "pe":
                sum_ps = sum_tiles[0]
                if oi == 0:
                    sum_ps = ps_acc.tile([128, 512], F32, tag="sum",
                                         name="sum_ps")
                    sum_tiles[0] = sum_ps
                nc.tensor.matmul(
                    sum_ps, lhsT=ones, rhs=pts,
                    start=(oi == 0), stop=(oi == nblk - 1),
                )
        if SUMS_MODE == "pe":
            qb_ = i % QBC
            rb = rb_pool.tile([128, 512], F32, tag="rb", name="rb")
            nc.vector.reciprocal(rb, sum_tiles[0])
            aTc2 = aTs[(i // QBC) % 2]
            nc.vector.tensor_mul(
                aTc2[:, qb_ * 512 : (qb_ + 1) * 512], O_ps, rb)
        else:
            pending[0] = (i, folded, O_ps)

    # ---- merged pipeline ----
    grps = _load_chunk(0, first=True)
    for sc in range(NSC):
        next_grps = _load_chunk(sc + 1) if sc + 1 < NSC else None
        vt_sb = _proj_chunk(sc, grps)
        grps = next_grps
        if sc >= 1:
            g = sc - 1
            aTs[g % 2] = aT_pool.tile(
                [128, QH * SCHUNK], BF16, tag="aT", name=f"aT_{g}")
            for qb in range(QBC):
                _attn_qblock(g * QBC + qb)
        _v_transpose(sc, vt_sb)
    g = NSC - 1
    aTs[g % 2] = aT_pool.tile([128, QH * SCHUNK], BF16, tag="aT", name=f"aT_{g}")
    for qb in range(QBC):
        _attn_qblock(g * QBC + qb)
    _finish_pending()
    _oproj(NB - 2)
    _oproj(NB - 1)
    st.close()


def build_kernel(nc, reps=1):
    hsT = nc.dram_tensor("hsT", [HID, S], BF16, kind="ExternalInput").ap()
    wq = nc.dram_tensor("wq", [HID, QH * D], BF16, kind="ExternalInput").ap()
    wk = nc.dram_tensor("wk", [HID, D], BF16, kind="ExternalInput").ap()
    wv = nc.dram_tensor("wv", [HID, D], BF16, kind="ExternalInput").ap()
    wo = nc.dram_tensor("wo", [QH * D, HID], BF16, kind="ExternalInput").ap()
    cos2 = nc.dram_tensor("cos2", [128, S], BF16, kind="ExternalInput").ap()
    sin2 = nc.dram_tensor("sin2", [128, S], BF16, kind="ExternalInput").ap()
    out_p = nc.dram_tensor("out_p", [S, HID], BF16, kind="ExternalOutput").ap()
    aps = (hsT, wq, wk, wv, wo, cos2, sin2, out_p)

    with tile.TileContext(nc) as tc:
        with tc.tile_pool(name="persist", bufs=1) as persist:
            for _rep in range(reps):
                _emit_body(nc, tc, persist, aps)
    return nc


_NC = {}


def _get_nc(reps=1):
    if reps not in _NC:
        nc = bacc.Bacc(
            "TRN2", target_bir_lowering=False, debug=False, num_devices=N_CORES
        )
        build_kernel(nc, reps=reps)
        nc.compile()
        _NC[reps] = nc
    return _NC[reps]


def make_exec_fn(nc, n_cores=N_CORES):
    """Build a reusable sharded executor for a compiled Bass module.

    Mirrors bass2jax.run_bass_via_pjrt's multi-core branch, but without
    donation so the zero output buffers can stay device-resident across
    repeated calls (for benchmarking).
    """
    import jax
    from jax.sharding import Mesh, NamedSharding, PartitionSpec
    from jax.experimental.shard_map import shard_map

    from concourse import bass2jax

    bass2jax.install_neuronx_cc_hook()

    partition_name = nc.partition_id_tensor.name if nc.partition_id_tensor else None
    in_names, out_names, out_avals, zero_outs = [], [], [], []
    for alloc in nc.m.functions[0].allocations:
        if not isinstance(alloc, mybir.MemoryLocationSet):
            continue
        name = alloc.memorylocations[0].name
        if alloc.kind == "ExternalInput":
            if name != partition_name:
                in_names.append(name)
        elif alloc.kind == "ExternalOutput":
            out_names.append(name)
            shape = tuple(alloc.tensor_shape)
            dtype = mybir.dt.np(alloc.dtype)
            out_avals.append(jax.core.ShapedArray(shape, dtype))
            zero_outs.append(np.zeros(shape, dtype))
    all_in_names = list(in_names) + list(out_names)
    if partition_name is not None:
        all_in_names.append(partition_name)
    all_in_names = tuple(all_in_names)

    def _body(*args):
        operands = list(args)
        if partition_name is not None:
            operands.append(bass2jax.partition_id_tensor())
        outs = bass2jax._bass_exec_p.bind(
            *operands,
            out_avals=tuple(out_avals),
            in_names=all_in_names,
            out_names=tuple(out_names),
            lowering_input_output_aliases=(),
            sim_require_finite=True,
            sim_require_nnan=True,
            nc=nc,
        )
        return tuple(outs)

    devices = jax.devices()[:n_cores]
    mesh = Mesh(np.asarray(devices), ("core",))
    spec = PartitionSpec("core")
    in_specs = (spec,) * (len(in_names) + len(out_names))
    out_specs = (spec,) * len(out_names)
    fn = jax.jit(
        shard_map(
            _body, mesh=mesh, in_specs=in_specs, out_specs=out_specs, check_rep=False
        ),
        keep_unused=True,
    )
    return fn, in_names, out_names, zero_outs, NamedSharding(mesh, spec)


_EXEC = None


def _get_exec():
    global _EXEC
    if _EXEC is None:
        _EXEC = make_exec_fn(_get_nc())
    return _EXEC


def _concat_args(in_maps, in_names, zero_outs):
    concat_in = [
        np.concatenate([np.asarray(in_maps[c][nm]) for c in range(N_CORES)], axis=0)
        for nm in in_names
    ]
    concat_zeros = [
        np.zeros((N_CORES * z.shape[0], *z.shape[1:]), z.dtype) for z in zero_outs
    ]
    return concat_in + concat_zeros


def _host_inputs(hidden_states, wq, wk, wv, wo):
    hs = np.asarray(hidden_states, np.float32).reshape(S, HID)
    hsT = np.ascontiguousarray(hs.T).astype(NPBF)

    scale = 1.0 / math.sqrt(D)
    inv_freq = 1.0 / (THETA ** (np.arange(0, D, 2, dtype=np.float32) / D))
    t = np.arange(S, dtype=np.float32)
    freqs = np.outer(t, inv_freq)                      # [S, 64]
    cosT = np.cos(freqs).T.astype(np.float32)          # [64, S]
    sinT = np.sin(freqs).T.astype(np.float32)
    cos2 = np.ascontiguousarray(np.concatenate([cosT, cosT], 0)).astype(NPBF)
    sin2 = np.ascontiguousarray(np.concatenate([-sinT, sinT], 0)).astype(NPBF)

    wq = np.asarray(wq, np.float32) * scale
    in_maps = []
    for c in range(N_CORES):
        in_maps.append(
            {
                "hsT": hsT,
                "wq": np.ascontiguousarray(wq[:, c * 512 : (c + 1) * 512]).astype(NPBF),
                "wk": np.ascontiguousarray(
                    np.asarray(wk, np.float32)[:, c * 128 : (c + 1) * 128]
                ).astype(NPBF),
                "wv": np.ascontiguousarray(
                    np.asarray(wv, np.float32)[:, c * 128 : (c + 1) * 128]
                ).astype(NPBF),
                "wo": np.ascontiguousarray(
                    np.asarray(wo, np.float32)[c * 512 : (c + 1) * 512, :]
                ).astype(NPBF),
                "cos2": cos2,
                "sin2": sin2,
            }
        )
    return in_maps


def _reduce_out(out_concat):
    acc = (
        np.asarray(out_concat)
        .reshape(N_CORES, S, HID)
        .astype(np.float32)
        .sum(axis=0)
    )
    return np.ascontiguousarray(acc).reshape(1, S, HID)


def run(hidden_states, wq, wk, wv, wo):
    """Returns full fp32 output [1, S, HID]."""
    import jax

    fn, in_names, out_names, zero_outs, sh = _get_exec()
    in_maps = _host_inputs(hidden_states, wq, wk, wv, wo)
    args = _concat_args(in_maps, in_names, zero_outs)
    outs = jax.block_until_ready(fn(*args))
    return _reduce_out(outs[0])


def bench(hidden_states, wq, wk, wv, wo, iters=10):
    """Repeated device-resident executions; returns (out, per-iter seconds)."""
    import time

    import jax

    fn, in_names, out_names, zero_outs, sh = _get_exec()
    in_maps = _host_inputs(hidden_states, wq, wk, wv, wo)
    args = _concat_args(in_maps, in_names, zero_outs)
    dev_args = jax.block_until_ready([jax.device_put(a, sh) for a in args])
    outs = jax.block_until_ready(fn(*dev_args))  # warm-up + compile
    times = []
    for _ in range(iters):
        t0 = time.perf_counter()
        o = fn(*dev_args)
        jax.block_until_ready(o)
        times.append(time.perf_counter() - t0)
    for n in (1, iters):
        t0 = time.perf_counter()
        os_ = [fn(*dev_args) for _ in range(n)]
        jax.block_until_ready(os_)
        times.append((time.perf_counter() - t0) / n)
    return _reduce_out(outs[0]), times


def kernel(hidden_states, wq, wk, wv, wo):
    return run(hidden_states, wq, wk, wv, wo)
